# revision 27
# baseline (speedup 1.0000x reference)
"""Trainium2 Bass kernel for nn_PolicyNet_78365973283198 (GNN message passing).

Computation (reference):
    tempHS = tanh(state_HS @ W_fs + b_fs)          # [N, 128]
    u0     = tempHS @ W_fp + b_fp                  # [N]
    uk[e]  = <tempHS[seg[e]], hats[e]>             # [E]  (seg sorted)
    out    = sigmoid(concat([u0, uk]))             # [N + E]

v2 strategy: data-parallel over nodes on 8 NeuronCores; each core owns 6250
contiguous nodes and their (contiguous, seg-sorted) edges. Per 128-node block
k the device computes, entirely in "transposed" orientation:

  tpT [d, n]  = sum_ch W_fs_chunk^T @ state_chunk      (PE, fp16, 4 matmuls)
  thT [d, n]  = tanh(tpT + b_fs)                       (ACT, per-partition bias)
  u0 col k    = thT^T @ W_fp                           (PE -> PSUM column k)
  per 512-edge chunk c:
    P   [n, e] = thT^T @ hatsT_chunk                   (PE matmul, fp16)
    M   [n, e] = P * onehot_chunk                      (DVE, fp8 mask, fp16 out)
    per 128-edge tile: uk col = M_tile^T @ ones        (PE matmul -> PSUM column)

The one-hot mask (seg[e] == n) is precomputed on host and uploaded as fp8
(exact 0/1), so no GPSIMD work and no on-device broadcast of seg is needed.
uk/u0 accumulate as PSUM columns (matmul outputs must start at partition 0,
so column-shaped N=1 outputs are used instead of row writes); one sigmoid
pass each at the end. M in fp16 is safe: each uk column has exactly one
active element, so no rounding accumulation occurs.
"""

import numpy as np

# Problem dims (hardcoded per spec; kernel.py must be self-contained).
N_NODES = 50000
N_EDGES = 600000
IN_DIM = 512
E_DIM = 128
NC = 8
NPER = N_NODES // NC            # 6250 nodes per core
KN = (NPER + 127) // 128        # 49 node tiles per core
NPAD = KN * 128                 # 6272
SCH = IN_DIM // 128             # 4 s-chunks
CW = 512                        # edge chunk width (one PSUM bank of f32)


def _chunks(m):
    """Chunk (offset, width) list covering m*128 edge columns."""
    out = []
    off = 0
    while off < m * 128:
        w = min(CW, m * 128 - off)
        out.append((off, w))
        off += w
    return out


def build_nc(m, reps=1):
    """Build the SPMD Bass program. m = max edge tiles per 128-node block."""
    import concourse.bass as bass
    import concourse.tile as tile
    from concourse import mybir

    fp16 = mybir.dt.float16
    fp8 = mybir.dt.float8e4
    f32 = mybir.dt.float32
    Alu = mybir.AluOpType
    Act = mybir.ActivationFunctionType

    nc = bass.Bass("TRN2", target_bir_lowering=False, debug=False)

    ME = m * 128                       # padded edges per block
    CHS = _chunks(m)                   # chunk list
    NCH = len(CHS)
    G = KN * m                         # total uk columns (one per edge tile)
    KB = (G + CW - 1) // CW            # uk PSUM banks needed (<= 2)
    assert KB <= 2, (m, G)

    # Inputs (per core)
    st_d = nc.dram_tensor("st_p", [KN, 128, SCH * 128], fp16, kind="ExternalInput")
    hats_d = nc.dram_tensor("hats_p", [KN, 128, ME], fp16, kind="ExternalInput")
    wcat_d = nc.dram_tensor("wcat_p", [128, SCH * 128], fp16, kind="ExternalInput")
    bfs_d = nc.dram_tensor("bfs_p", [128, 1], f32, kind="ExternalInput")
    wfp_d = nc.dram_tensor("wfp_p", [128, 1], fp16, kind="ExternalInput")
    bfp_d = nc.dram_tensor("bfp_p", [128, 1], f32, kind="ExternalInput")
    oh_d = nc.dram_tensor("oh_p", [KN, 128, ME], fp8, kind="ExternalInput")
    u0_d = nc.dram_tensor("u0_o", [128, KN], f32, kind="ExternalOutput")
    uk_d = nc.dram_tensor("uk_o", [128, G], f32, kind="ExternalOutput")

    with tile.TileContext(nc) as tc:
        with (
            tc.tile_pool(name="const", bufs=1) as cpool,
            tc.tile_pool(name="perst", bufs=1) as ppool,
            tc.tile_pool(name="st", bufs=3) as spool,
            tc.tile_pool(name="hat", bufs=3) as hpool,
            tc.tile_pool(name="oh", bufs=3) as opool,
            tc.tile_pool(name="th", bufs=3) as thpool,
            tc.tile_pool(name="mm", bufs=5) as mpool,
            tc.tile_pool(name="psT", bufs=2, space="PSUM") as psT,
            tc.tile_pool(name="psP", bufs=3, space="PSUM") as psP,
            tc.tile_pool(name="psUV", bufs=1, space="PSUM") as psUV,
        ):
            # --- constants ---
            wcat = cpool.tile([128, SCH * 128], fp16, tag="wcat")
            nc.sync.dma_start(wcat[:], wcat_d[:])
            bfs = cpool.tile([128, 1], f32, tag="bfs")
            nc.sync.dma_start(bfs[:], bfs_d[:])
            wfp = cpool.tile([128, 1], fp16, tag="wfp")
            nc.sync.dma_start(wfp[:], wfp_d[:])
            bfp = cpool.tile([128, 1], f32, tag="bfp")
            nc.sync.dma_start(bfp[:], bfp_d[:])
            ones = cpool.tile([128, 1], fp16, tag="ones")
            nc.vector.memset(ones[:], 1.0)

            # persistent PSUM accumulators (columns written by N=1 matmuls)
            u0t = psUV.tile([128, 128], f32, tag="u0t")
            ukt = psUV.tile([128, KB * CW], f32, tag="ukt")
            # persistent SBUF staging for outputs
            u0s = ppool.tile([128, KN], f32, tag="u0s")
            uks = ppool.tile([128, G], f32, tag="uks")

            for rep in range(reps):
                for k in range(KN):
                    st = spool.tile([128, SCH * 128], fp16, tag="st")
                    nc.sync.dma_start(st[:], st_d[k])
                    hat = hpool.tile([128, ME], fp16, tag="hat")
                    nc.scalar.dma_start(hat[:], hats_d[k])
                    ohb = opool.tile([128, ME], fp8, tag="ohb")
                    nc.gpsimd.dma_start(ohb[:], oh_d[k])

                    # FS: tpT[d, n] accumulated over 4 state chunks
                    tp = psT.tile([128, 128], f32, tag="tp")
                    for ch in range(SCH):
                        nc.tensor.matmul(
                            tp[:],
                            lhsT=wcat[:, ch * 128:(ch + 1) * 128],
                            rhs=st[:, ch * 128:(ch + 1) * 128],
                            start=(ch == 0), stop=(ch == SCH - 1),
                        )
                    th = thpool.tile([128, 128], fp16, tag="th")
                    nc.scalar.activation(th[:], tp[:], Act.Tanh, bias=bfs[:])
                    # u0 column: u0[n] = sum_d thT[d, n] * wfp[d]
                    nc.tensor.matmul(u0t[:, k:k + 1], lhsT=th[:],
                                     rhs=wfp[:], start=True, stop=True)
                    # edge chunks: P matmul, mask multiply (DVE), reduce (PE).
                    # Reduces are emitted one chunk late so the PE doesn't
                    # stall on the DVE multiply it just enabled.
                    pend = None      # (off, w, Mb) awaiting reduction
                    for c, (off, w) in enumerate(CHS):
                        Pp = psP.tile([128, CW], f32, tag="Pp")
                        nc.tensor.matmul(Pp[:, :w], lhsT=th[:],
                                         rhs=hat[:, off:off + w],
                                         start=True, stop=True)
                        Mb = mpool.tile([128, CW], fp16, tag="Mb")
                        nc.vector.tensor_tensor(
                            out=Mb[:, :w], in0=Pp[:, :w],
                            in1=ohb[:, off:off + w], op=Alu.mult)
                        if pend is not None:
                            po, pw_, pmb = pend
                            for t in range(pw_ // 128):
                                g = k * m + (po // 128) + t
                                nc.tensor.matmul(
                                    ukt[:, g:g + 1],
                                    lhsT=pmb[:, t * 128:(t + 1) * 128],
                                    rhs=ones[:], start=True, stop=True)
                        pend = (off, w, Mb)
                    po, pw_, pmb = pend
                    for t in range(pw_ // 128):
                        g = k * m + (po // 128) + t
                        nc.tensor.matmul(
                            ukt[:, g:g + 1],
                            lhsT=pmb[:, t * 128:(t + 1) * 128],
                            rhs=ones[:], start=True, stop=True)

            # ---- outputs ----
            nc.scalar.activation(u0s[:], u0t[:, 0:KN], Act.Sigmoid,
                                 bias=bfp[:])
            nc.sync.dma_start(u0_d[:], u0s[:])
            nc.scalar.activation(uks[:], ukt[:, 0:G], Act.Sigmoid)
            nc.sync.dma_start(uk_d[:], uks[:])

    split_multi_waits(nc)
    return nc


def split_multi_waits(nc):
    """This env's walrus encodes at most one sem wait per instruction; hoist
    extras onto standalone EventSemaphore insts immediately before."""
    import concourse.mybir as mybir
    n = 0
    for fn in nc.m.functions:
        for bb in fn.blocks:
            insts = list(bb.instructions)
            if not any(i.sync_info and len(i.sync_info.on_wait) > 1 for i in insts):
                continue
            out = []
            for inst in insts:
                si = inst.sync_info
                if si is not None and len(si.on_wait) > 1:
                    waits = list(si.on_wait)
                    for w in waits[:-1]:
                        n += 1
                        out.append(mybir.InstEventSemaphore(
                            name=f"splitw_{n}_{inst.name}",
                            engine=inst.engine, ins=[], outs=[],
                            sync_info=mybir.SyncInfo(on_wait=[w], on_update=[]),
                        ))
                    inst.sync_info = mybir.SyncInfo(
                        on_wait=[waits[-1]], on_update=list(si.on_update))
                out.append(inst)
            bb.instructions = out
    return n


def prep_inputs(state_HS, hats, seg, W_fs, b_fs, W_fp, b_fp):
    """Shard + pack full inputs into per-core in_maps. Returns
    (in_maps, m, blk_counts, blk_starts)."""
    import ml_dtypes
    fp8np = ml_dtypes.float8_e4m3fn

    state_HS = np.asarray(state_HS, dtype=np.float32)
    hats = np.asarray(hats, dtype=np.float32)
    seg = np.asarray(seg, dtype=np.int32)
    W_fs = np.asarray(W_fs, dtype=np.float32)
    b_fs = np.asarray(b_fs, dtype=np.float32)
    W_fp = np.asarray(W_fp, dtype=np.float32)
    b_fp = np.asarray(b_fp, dtype=np.float32)

    # per-(core, block) edge ranges; blocks are 128-node groups
    bounds = np.empty((NC, KN + 1), dtype=np.int64)
    for c in range(NC):
        n0 = c * NPER
        ids = np.minimum(n0 + np.arange(KN + 1) * 128, n0 + NPER)
        bounds[c] = np.searchsorted(seg, ids)
    blk_counts = np.diff(bounds, axis=1)          # [NC, KN]
    blk_starts = bounds[:, :-1]                   # [NC, KN]
    m = max(1, int(np.ceil(blk_counts.max() / 128)))
    ME = m * 128

    # shared weights
    # wcat[s_loc, ch*128 + d] = W_fs[ch*128 + s_loc, d]
    wcat = np.ascontiguousarray(
        W_fs.reshape(SCH, 128, E_DIM).transpose(1, 0, 2).reshape(128, SCH * 128)
    ).astype(np.float16)
    bfs_col = b_fs.reshape(128, 1).astype(np.float32)
    wfp_col = W_fp.reshape(128, 1).astype(np.float16)
    bfp_col = np.full((128, 1), float(b_fp[0]), dtype=np.float32)

    in_maps = []
    for c in range(NC):
        n0 = c * NPER
        stf = np.zeros((NPAD, IN_DIM), dtype=np.float32)
        stf[:NPER] = state_HS[n0:n0 + NPER]

        # st_p[k, s_loc, ch*128 + n] = state[k*128 + n, ch*128 + s_loc]
        stp = np.ascontiguousarray(
            stf.reshape(KN, 128, SCH, 128).transpose(0, 3, 2, 1)
        ).reshape(KN, 128, SCH * 128).astype(np.float16)

        hp = np.zeros((KN, ME, E_DIM), dtype=np.float32)
        ohp = np.zeros((KN, 128, ME), dtype=fp8np)
        for k in range(KN):
            cnt = int(blk_counts[c, k])
            s0 = int(blk_starts[c, k])
            hp[k, :cnt] = hats[s0:s0 + cnt]
            rows = (seg[s0:s0 + cnt] - n0 - k * 128).astype(np.int64)
            ohp[k, rows, np.arange(cnt)] = 1.0
        # hats_p[k, d, e] = hp[k, e, d]
        hpT = np.ascontiguousarray(hp.transpose(0, 2, 1)).astype(np.float16)

        in_maps.append({
            "st_p": stp, "wcat_p": wcat, "bfs_p": bfs_col,
            "wfp_p": wfp_col, "bfp_p": bfp_col,
            "hats_p": hpT, "oh_p": ohp,
        })
    return in_maps, m, blk_counts, blk_starts


def assemble(results, m, blk_counts, blk_starts):
    out = np.empty(N_NODES + N_EDGES, dtype=np.float32)
    for c in range(NC):
        u0 = np.asarray(results[c]["u0_o"])       # [128, KN] (cols = k)
        uk = np.asarray(results[c]["uk_o"])       # [128, KN*m] (cols = tiles)
        out[c * NPER:(c + 1) * NPER] = u0.T.reshape(-1)[:NPER]
        for k in range(KN):
            cnt = int(blk_counts[c, k])
            s0 = int(blk_starts[c, k])
            for t in range((cnt + 127) // 128):
                w = min(128, cnt - t * 128)
                g = k * m + t
                out[N_NODES + s0 + t * 128:N_NODES + s0 + t * 128 + w] = \
                    uk[:w, g]
    return out


def kernel(state_HS, hats, seg, W_fs, b_fs, W_fp, b_fp):
    from concourse.bass_utils import run_bass_kernel_spmd
    in_maps, m, blk_counts, blk_starts = prep_inputs(
        state_HS, hats, seg, W_fs, b_fs, W_fp, b_fp)
    nc = build_nc(m)
    res = run_bass_kernel_spmd(nc, in_maps, core_ids=list(range(NC)))
    return assemble(res.results, m, blk_counts, blk_starts)
